# revision 6
# baseline (speedup 1.0000x reference)
"""Trainium2 Bass kernel for nn_PositionalEncoding_3341484556841.

Computes, for user nodes 0..4095 of an 8192-node dense graph:
  degrees = adj.sum(1)                      (row sums)
  pr      = pagerank(adj)                   (power iteration, converges in 2 updates)
  out     = mlp2(concat(x, spe_mlp(0), de_mlp(deg), pre_mlp(pr)))

Distribution: adj is row-sharded over 8 NeuronCores (1024 rows each, fp32 HBM ->
bf16 SBUF-resident via cast-DMA, one pass = the memory roofline).  Each PageRank
update is a local matvec against the resident shard producing a full-length
partial, exchanged with a ReduceScatter (each core receives its own summed
slice).  Row sums stay local under row sharding.  The small per-node MLPs run
data-parallel, core c handling nodes [1024c, 1024(c+1)); cores 0-3 cover all
4096 users and only their outputs are gathered.  All MLP math is fp32 on the PE;
only the PageRank matvec (whose contribution to the output is ~1e-8 relative)
uses bf16.
"""

import numpy as np

import concourse.bass as bass
import concourse.mybir as mybir
import concourse.tile as tile
from concourse import bacc
from concourse.bass_utils import run_bass_kernel_spmd

N_CORES = 8
N = 8192            # total nodes
R = 1024            # adj rows per core
NT = R // 128       # 8 row tiles of [128, N] per core
NU = 4096           # users
L = 128             # latent
Q = 32
DAMP = 0.85
N_UPDATES = 2       # reference early-stops after exactly 2 applied updates
F32 = mybir.dt.float32
BF16 = mybir.dt.bfloat16

_CACHED = {}


def build():
    nc = bacc.Bacc("TRN2", target_bir_lowering=False, debug=False, num_devices=N_CORES)

    adj = nc.dram_tensor("adj", [R, N], F32, kind="ExternalInput").ap()
    xT = nc.dram_tensor("xT", [L, R], F32, kind="ExternalInput").ap()
    cw1a = nc.dram_tensor("cw1a", [128, L], F32, kind="ExternalInput").ap()
    cw1b = nc.dram_tensor("cw1b", [3 * Q, L], F32, kind="ExternalInput").ap()
    cb1 = nc.dram_tensor("cb1", [L, 1], F32, kind="ExternalInput").ap()
    cw2 = nc.dram_tensor("cw2", [L, L], F32, kind="ExternalInput").ap()
    cb2 = nc.dram_tensor("cb2", [L, 1], F32, kind="ExternalInput").ap()
    # small mlps: [w1 (1,Q)]; b1 (Q,1); w2 (Q,Q); b2 (Q,1)
    de_w1 = nc.dram_tensor("de_w1", [1, Q], F32, kind="ExternalInput").ap()
    de_b1 = nc.dram_tensor("de_b1", [Q, 1], F32, kind="ExternalInput").ap()
    de_w2 = nc.dram_tensor("de_w2", [Q, Q], F32, kind="ExternalInput").ap()
    de_b2 = nc.dram_tensor("de_b2", [Q, 1], F32, kind="ExternalInput").ap()
    pre_w1 = nc.dram_tensor("pre_w1", [1, Q], F32, kind="ExternalInput").ap()
    pre_b1 = nc.dram_tensor("pre_b1", [Q, 1], F32, kind="ExternalInput").ap()
    pre_w2 = nc.dram_tensor("pre_w2", [Q, Q], F32, kind="ExternalInput").ap()
    pre_b2 = nc.dram_tensor("pre_b2", [Q, 1], F32, kind="ExternalInput").ap()
    spe_b1 = nc.dram_tensor("spe_b1", [Q, 1], F32, kind="ExternalInput").ap()
    spe_w2 = nc.dram_tensor("spe_w2", [Q, Q], F32, kind="ExternalInput").ap()
    spe_b2 = nc.dram_tensor("spe_b2", [Q, 1], F32, kind="ExternalInput").ap()

    outT = nc.dram_tensor("outT", [L, R], F32, kind="ExternalOutput").ap()
    dbg = nc.dram_tensor("dbg", [1, 8], F32, kind="ExternalOutput").ap()
    DEBUG = _CACHED.get("debug", False)
    if DEBUG:
        d_degrow = nc.dram_tensor("d_degrow", [1, R], F32, kind="ExternalOutput").ap()
        d_prow = nc.dram_tensor("d_prow", [1, N], F32, kind="ExternalOutput").ap()
        d_prslice = nc.dram_tensor("d_prslice", [1, R], F32, kind="ExternalOutput").ap()
        d_feats = nc.dram_tensor("d_feats", [3 * Q, R], F32, kind="ExternalOutput").ap()
        d_hidc = nc.dram_tensor("d_hidc", [L, R], F32, kind="ExternalOutput").ap()

    rg = [list(range(N_CORES))]

    with tile.TileContext(nc) as tc:
        with (
            tc.tile_pool(name="res", bufs=1) as res_pool,
            tc.tile_pool(name="sb", bufs=1) as sb,
            tc.tile_pool(name="mv", bufs=4, space="PSUM") as mv_psum,
            tc.tile_pool(name="tp", bufs=3, space="PSUM") as tail_psum,
            tc.tile_pool(name="dram", bufs=1, space="DRAM") as dram,
        ):
            # ---- dummy warmup collective: absorbs first-collective setup cost
            # under the adj load (collectives run on TOPSP/SDMA, engines idle)
            warm_sb = sb.tile([1, 8], F32, name="warm_sb")
            nc.vector.memset(warm_sb[:], 0.0)
            warm_in = dram.tile([1, 8], F32, name="warm_in")
            warm_out = dram.tile([1, 8], F32, name="warm_out", addr_space="Shared")
            nc.sync.dma_start(out=warm_in[:], in_=warm_sb[:])
            nc.gpsimd.collective_compute(
                "AllReduce", mybir.AluOpType.add, replica_groups=rg,
                ins=[warm_in.opt()], outs=[warm_out.opt()],
            )
            nc.sync.dma_start(out=dbg[:], in_=warm_out[:])

            # ---- load adj shard: fp32 DRAM -> bf16 SBUF resident (cast DMA)
            a_tiles = [res_pool.tile([128, N], BF16, name=f"a{t}") for t in range(NT)]
            NDMA = 4
            chunk = N // NDMA
            for t in range(NT):
                for h in range(NDMA):
                    nc.gpsimd.dma_start(
                        out=a_tiles[t][:, h * chunk:(h + 1) * chunk],
                        in_=adj[t * 128:(t + 1) * 128, h * chunk:(h + 1) * chunk],
                    )

            # ---- row sums (raw degrees) from bf16 resident, fp32 accumulate
            deg = sb.tile([128, NT], F32, name="deg")
            part4 = sb.tile([128, 4], F32, name="part4")
            for t in range(NT):
                for s in range(4):
                    nc.vector.reduce_sum(
                        part4[:, s:s + 1], a_tiles[t][:, s * 2048:(s + 1) * 2048],
                        axis=mybir.AxisListType.X)
                nc.vector.reduce_sum(deg[:, t:t + 1], part4[:], axis=mybir.AxisListType.X)

            # clipped reciprocal for pagerank scaling
            degc = sb.tile([128, NT], F32, name="degc")
            nc.vector.tensor_scalar_max(degc[:], deg[:], 1.0)
            recip = sb.tile([128, NT], F32, name="recip")
            nc.vector.reciprocal(recip[:], degc[:])

            # identity (for PE-based column->row moves) and ones[1,1]
            ident = sb.tile([128, 128], F32, name="ident")
            from concourse.masks import make_identity
            make_identity(nc, ident[:])
            one11 = sb.tile([1, 1], F32, name="one11")
            nc.vector.memset(one11[:], 1.0)

            # deg as a node-ordered row [1, R] via PE: deg[:,t].T @ I
            deg_row = sb.tile([1, R], F32, name="deg_row")
            for h in range(2):
                drps = tail_psum.tile([1, 512], F32, name=f"drps_{h}", tag="tp")
                for tt in range(4):
                    t = h * 4 + tt
                    nc.tensor.matmul(drps[:, tt * 128:(tt + 1) * 128],
                                     deg[:, t:t + 1], ident[:], start=True, stop=True)
                nc.scalar.copy(deg_row[:, h * 512:(h + 1) * 512], drps[:])

            # ---- pagerank: v0 = (1/N) * recip ; two matvec+ReduceScatter updates
            vcur = sb.tile([128, NT], BF16, name="vcur")
            vf = sb.tile([128, NT], F32, name="vf")
            nc.vector.tensor_scalar_mul(vf[:], recip[:], 1.0 / N)
            nc.vector.tensor_copy(vcur[:], vf[:])

            prow = sb.tile([1, N], F32, name="prow")
            pr_slice = sb.tile([1, R], F32, name="pr_slice")

            for it in range(N_UPDATES):
                # partial[j] = sum_k v[k] * A[k, j], four 4-bank sweeps over N
                for s in range(4):
                    ps_tiles = []
                    for nb in range(4):
                        ps = mv_psum.tile([1, 512], F32, name=f"ps_{it}_{s}_{nb}", tag="mv")
                        ps_tiles.append(ps)
                        col0 = (s * 4 + nb) * 512
                        for t in range(NT):
                            nc.tensor.matmul(
                                ps[:], vcur[:, t:t + 1],
                                a_tiles[t][:, col0:col0 + 512],
                                start=(t == 0), stop=(t == NT - 1))
                    for nb in range(4):
                        col0 = (s * 4 + nb) * 512
                        # fold pr update: pr = d * sum(partial) + (1-d)/N
                        # ReduceScatter sums 8 buffers -> add (1-d)/(8N) in each
                        nc.vector.tensor_scalar(
                            out=prow[:, col0:col0 + 512], in0=ps_tiles[nb][:],
                            scalar1=DAMP, scalar2=(1.0 - DAMP) / (8.0 * N),
                            op0=mybir.AluOpType.mult, op1=mybir.AluOpType.add)

                rs_in = dram.tile([1, N], F32, name=f"rs_in_{it}")
                rs_out = dram.tile([1, R], F32, name=f"rs_out_{it}")
                nc.sync.dma_start(out=rs_in[:], in_=prow[:])
                nc.gpsimd.collective_compute(
                    "ReduceScatter", mybir.AluOpType.add, replica_groups=rg,
                    ins=[rs_in.rearrange("o (c r) -> (o c) r", c=N_CORES).opt()],
                    outs=[rs_out.opt()],
                )
                nc.sync.dma_start(out=pr_slice[:], in_=rs_out[:])

                if it < N_UPDATES - 1:
                    # v_next[p, t] = pr_slice[128t + p] * recip[p, t]
                    pv = tail_psum.tile([128, NT], F32, name=f"pv_{it}", tag="tp")
                    for t in range(NT):
                        nc.tensor.matmul(pv[:, t:t + 1],
                                         pr_slice[:, t * 128:(t + 1) * 128], one11[:],
                                         start=True, stop=True)
                    nc.vector.tensor_mul(vf[:], pv[:], recip[:])
                    nc.vector.tensor_copy(vcur[:], vf[:])

            # ---- tail MLPs (fp32), node-parallel over this core's R nodes
            w_de1 = sb.tile([1, Q], F32, name="w_de1")
            b_de1 = sb.tile([Q, 1], F32, name="b_de1")
            w_de2 = sb.tile([Q, Q], F32, name="w_de2")
            b_de2 = sb.tile([Q, 1], F32, name="b_de2")
            w_pre1 = sb.tile([1, Q], F32, name="w_pre1")
            b_pre1 = sb.tile([Q, 1], F32, name="b_pre1")
            w_pre2 = sb.tile([Q, Q], F32, name="w_pre2")
            b_pre2 = sb.tile([Q, 1], F32, name="b_pre2")
            b_spe1 = sb.tile([Q, 1], F32, name="b_spe1")
            w_spe2 = sb.tile([Q, Q], F32, name="w_spe2")
            b_spe2 = sb.tile([Q, 1], F32, name="b_spe2")
            for t_sb, t_dr in [(w_de1, de_w1), (b_de1, de_b1), (w_de2, de_w2), (b_de2, de_b2),
                               (w_pre1, pre_w1), (b_pre1, pre_b1), (w_pre2, pre_w2), (b_pre2, pre_b2),
                               (b_spe1, spe_b1), (w_spe2, spe_w2), (b_spe2, spe_b2)]:
                nc.sync.dma_start(out=t_sb[:], in_=t_dr[:])
            xT_sb = sb.tile([L, R], F32, name="xT_sb")
            nc.sync.dma_start(out=xT_sb[:], in_=xT[:])
            w_c1a = sb.tile([128, L], F32, name="w_c1a")
            w_c1b = sb.tile([3 * Q, L], F32, name="w_c1b")
            b_c1 = sb.tile([L, 1], F32, name="b_c1")
            w_c2 = sb.tile([L, L], F32, name="w_c2")
            b_c2 = sb.tile([L, 1], F32, name="b_c2")
            for t_sb, t_dr in [(w_c1a, cw1a), (w_c1b, cw1b), (b_c1, cb1), (w_c2, cw2), (b_c2, cb2)]:
                nc.sync.dma_start(out=t_sb[:], in_=t_dr[:])

            feats = sb.tile([3 * Q, R], F32, name="feats")

            # spe: constant column relu(b1) @ w2 + b2, broadcast along nodes
            relu_b1 = sb.tile([Q, 1], F32, name="relu_b1")
            nc.scalar.activation(relu_b1[:], b_spe1[:], mybir.ActivationFunctionType.Relu)
            spe_ps = tail_psum.tile([Q, 1], F32, name="spe_ps", tag="tp")
            nc.tensor.matmul(spe_ps[:], w_spe2[:], relu_b1[:], start=True, stop=True)
            spe_col = sb.tile([Q, 1], F32, name="spe_col")
            nc.vector.tensor_add(spe_col[:], spe_ps[:], b_spe2[:])
            nc.vector.memset(feats[0:Q, :], 0.0)
            nc.vector.tensor_scalar_add(feats[0:Q, :], feats[0:Q, :], spe_col[:])

            # de / pre mlps: rows of feats
            def small_mlp(in_row, w1, b1, w2, b2, out_rows, nm):
                hid = sb.tile([Q, R], F32, name=f"hid_{nm}", tag="hid")
                for nb in range(R // 512):
                    cs = slice(nb * 512, (nb + 1) * 512)
                    hid_ps = tail_psum.tile([Q, 512], F32, name=f"hp_{nm}_{nb}", tag="tp")
                    nc.tensor.matmul(hid_ps[:], w1[:], in_row[:, cs], start=True, stop=True)
                    nc.scalar.activation(hid[:, cs], hid_ps[:],
                                         mybir.ActivationFunctionType.Relu, bias=b1[:])
                for nb in range(R // 512):
                    cs = slice(nb * 512, (nb + 1) * 512)
                    out_ps = tail_psum.tile([Q, 512], F32, name=f"op_{nm}_{nb}", tag="tp")
                    nc.tensor.matmul(out_ps[:], w2[:], hid[:, cs], start=True, stop=True)
                    nc.vector.tensor_scalar_add(out_rows[:, cs], out_ps[:], b2[:])

            small_mlp(deg_row, w_de1, b_de1, w_de2, b_de2, feats[Q:2 * Q, :], "de")
            small_mlp(pr_slice, w_pre1, b_pre1, w_pre2, b_pre2, feats[2 * Q:3 * Q, :], "pre")

            if DEBUG:
                nc.sync.dma_start(out=d_degrow[:], in_=deg_row[:])
                nc.sync.dma_start(out=d_prow[:], in_=prow[:])
                nc.sync.dma_start(out=d_prslice[:], in_=pr_slice[:])
                nc.sync.dma_start(out=d_feats[:], in_=feats[:])

            # comb mlp: out.T = w2.T @ relu(w1.T @ [x; feats] + b1) + b2
            hidc = sb.tile([L, R], F32, name="hidc")
            for nb in range(R // 512):
                cs = slice(nb * 512, (nb + 1) * 512)
                h_ps = tail_psum.tile([L, 512], F32, name=f"cps_{nb}", tag="tp")
                nc.tensor.matmul(h_ps[:], w_c1a[:], xT_sb[:, cs], start=True, stop=False)
                nc.tensor.matmul(h_ps[:], w_c1b[:], feats[:, cs], start=False, stop=True)
                nc.scalar.activation(hidc[:, cs], h_ps[:],
                                     mybir.ActivationFunctionType.Relu, bias=b_c1[:])
            if DEBUG:
                nc.sync.dma_start(out=d_hidc[:], in_=hidc[:])
            for nb in range(R // 512):
                cs = slice(nb * 512, (nb + 1) * 512)
                o_ps = tail_psum.tile([L, 512], F32, name=f"ops_{nb}", tag="tp")
                nc.tensor.matmul(o_ps[:], w_c2[:], hidc[:, cs], start=True, stop=True)
                out_sb = sb.tile([L, 512], F32, name=f"out_sb_{nb}", tag="osb")
                nc.vector.tensor_scalar_add(out_sb[:], o_ps[:], b_c2[:])
                nc.sync.dma_start(out=outT[:, cs], in_=out_sb[:])

    nc.compile()
    return nc


def _prep_in_maps(inputs):
    adj = np.ascontiguousarray(np.asarray(inputs["adj"], dtype=np.float32))
    x = np.asarray(inputs["x"], dtype=np.float32)
    f32 = np.float32

    def col(v):
        return np.ascontiguousarray(np.asarray(v, dtype=f32).reshape(-1, 1))

    shared = {
        "cw1a": np.ascontiguousarray(np.asarray(inputs["comb_w1"], f32)[0:L, :]),
        "cw1b": np.ascontiguousarray(np.asarray(inputs["comb_w1"], f32)[L:L + 3 * Q, :]),
        "cb1": col(inputs["comb_b1"]),
        "cw2": np.ascontiguousarray(np.asarray(inputs["comb_w2"], f32)),
        "cb2": col(inputs["comb_b2"]),
        "de_w1": np.ascontiguousarray(np.asarray(inputs["de_w1"], f32)),
        "de_b1": col(inputs["de_b1"]),
        "de_w2": np.ascontiguousarray(np.asarray(inputs["de_w2"], f32)),
        "de_b2": col(inputs["de_b2"]),
        "pre_w1": np.ascontiguousarray(np.asarray(inputs["pre_w1"], f32)),
        "pre_b1": col(inputs["pre_b1"]),
        "pre_w2": np.ascontiguousarray(np.asarray(inputs["pre_w2"], f32)),
        "pre_b2": col(inputs["pre_b2"]),
        "spe_b1": col(inputs["spe_b1"]),
        "spe_w2": np.ascontiguousarray(np.asarray(inputs["spe_w2"], f32)),
        "spe_b2": col(inputs["spe_b2"]),
    }
    in_maps = []
    for c in range(N_CORES):
        m = dict(shared)
        m["adj"] = adj[c * R:(c + 1) * R, :]
        if c * R < NU:
            m["xT"] = np.ascontiguousarray(x[c * R:(c + 1) * R, :].T)
        else:
            m["xT"] = np.zeros((L, R), f32)
        in_maps.append(m)
    return in_maps


def kernel(**inputs) -> np.ndarray:
    if "nc" not in _CACHED:
        _CACHED["nc"] = build()
    nc = _CACHED["nc"]
    in_maps = _prep_in_maps(inputs)
    res = run_bass_kernel_spmd(nc, in_maps, core_ids=list(range(N_CORES)))
    node_indices = np.asarray(inputs["node_indices"]).astype(np.int64)
    out_users = np.concatenate(
        [res.results[c]["outT"].T for c in range(NU // R)], axis=0)  # [4096, 128]
    return np.ascontiguousarray(out_users[node_indices]).astype(np.float32)
